# revision 16
# baseline (speedup 1.0000x reference)
"""Balanced BCE loss on 8 Trainium2 NeuronCores.

loss = -sum_i [ beta_i * sum_j(t_ij * ln(p_ij))
                + (1-beta_i) * sum_j((1-t_ij) * ln(1-p_ij)) ]
beta_i = 1 - mean_j(t_ij)

Per-core slab layout: the core's 8 rows (8MB contiguous HBM) are viewed
as [128, 16384] f32 where partition p owns the contiguous 64KB line
slab[p*16384:(p+1)*16384]; row r <-> partitions 16r..16r+15.

Two independent DMA streams interleave at the SDMA engines, covering
each other's inter-transfer bubbles (measured near-gapless at line
rate ~370GB/s):
  p-chunks: HWDGE (nc.sync), f32
  t-chunks: SWDGE (nc.gpsimd) with f32->bf16 cast in the DMA datapath
            (kills the DVE cast op and halves t's SBUF footprint)

Streaming per column-chunk (DVE ops run 2x bf16 tensor_tensor mode;
the fused accum-reduce DVE variants only have 1x microcode and the PE
reduces for free, so all reductions go to the otherwise-idle PE):
  ACT: l1mp = Ln(1-p) bf16, accum_out -> accB[:,c]   (B for free)
       logp = Ln(p)  bf16
  DVE: m2 = t*l1mp, m1 = t*logp   (plain tensor_tensor, 2x)
  PE:  E^T @ {t, m2, m1} in <=512-col sub-blocks accumulated into
       psS/psC/psA [8,512] PSUM across all chunks (E = block-indicator
       [128,8], host-provided constant; E[p,r]=1 iff p//16==r)

The chunk schedule tapers (... 1024, 512, 256, 256) so the serial
dependency chain after the last DMA byte is short; the last chunk's
ops are reordered (logp/m1 first) and the PSUM folds are split between
ACT (psS, psA via Copy+accum, reads PSUM cheaply) and DVE (psC, psB)
so they overlap. Output padded to [8,128] f32 (512B per partition
descriptor - avoids the sub-512B DMA read-modify-write penalty).

host: loss = -sum_rows[ beta*A + (1-beta)*(B-C) ], beta = 1-S/N
"""

from contextlib import ExitStack

import numpy as np

import concourse.bass as bass
import concourse.mybir as mybir
import concourse.tile as tile
from concourse import bacc
from concourse.bass_utils import run_bass_kernel_spmd

B, N = 64, 262144
NCORES = 8
ROWS = B // NCORES  # rows per core
P = 128  # SBUF partitions
F = ROWS * N // P  # 16384 cols per partition
GRP = P // ROWS  # 16 partitions per row

# column-chunk schedule: sums to F; mid-stream big (2MB p-DMAs), tapered tail
CHUNKS = [2048, 4096, 4096, 2048, 2048, 1024, 512, 256, 256]
assert sum(CHUNKS) == F
NCH = len(CHUNKS)
CMAX = max(CHUNKS)
MM = 512  # matmul sub-block width (one PSUM bank)
OUTW = 128  # padded stats width: 512B per partition descriptor

AF = mybir.ActivationFunctionType
ALU = mybir.AluOpType
f32 = mybir.dt.float32
bf16 = mybir.dt.bfloat16

# test.py can flip this to capture an NTFF profile of the run
TRACE = False
LAST = None  # BassKernelResults of the most recent kernel() call


def _emit(tc, out_ap, inp_ap, tgt_ap, emat_ap):
    nc = tc.nc

    with ExitStack() as ctx:
        singles = ctx.enter_context(tc.tile_pool(name="const", bufs=1))
        pch_pool = ctx.enter_context(tc.tile_pool(name="pch", bufs=3))
        tch_pool = ctx.enter_context(tc.tile_pool(name="tch", bufs=3))
        ln_pool = ctx.enter_context(tc.tile_pool(name="ln", bufs=2))
        mm_pool = ctx.enter_context(tc.tile_pool(name="mm", bufs=2))
        psum_pool = ctx.enter_context(tc.tile_pool(name="ps", bufs=1, space="PSUM"))

        accB = singles.tile([P, NCH], f32, tag="accB")
        junkps = singles.tile([ROWS, MM], bf16, tag="junkps")
        accBr = singles.tile([P, 1], f32, tag="accBr")
        ematf = singles.tile([P, ROWS], f32, tag="ematf")
        ematb = singles.tile([P, ROWS], bf16, tag="ematb")
        stats = singles.tile([ROWS, OUTW], f32, tag="stats")
        psS = psum_pool.tile([ROWS, MM], f32, tag="psS", name="psS")
        psA = psum_pool.tile([ROWS, MM], f32, tag="psA", name="psA")
        psC = psum_pool.tile([ROWS, MM], f32, tag="psC", name="psC")
        psB = psum_pool.tile([ROWS, 1], f32, tag="psB", name="psB")

        nc.gpsimd.memset(stats[:], 0.0)

        # slab views: [rows, n] -> [128, F], 64KB contiguous per partition
        inp3 = inp_ap.rearrange("r (a f) -> (r a) f", a=GRP)
        tgt3 = tgt_ap.rearrange("r (a f) -> (r a) f", a=GRP)

        offs = [0]
        for c in CHUNKS:
            offs.append(offs[-1] + c)

        # all DMA triggers upfront: p on the SP HWDGE ring, t via SWDGE
        # with inline f32->bf16 cast
        ptiles, ttiles = [], []
        for c in range(NCH):
            o, e = offs[c], offs[c + 1]
            pt = pch_pool.tile([P, CMAX], f32, tag="p", name=f"p{c}")
            nc.sync.dma_start(pt[:, : e - o], inp3[:, o:e])
            ptiles.append(pt)
            tt = tch_pool.tile([P, CMAX], bf16, tag="t", name=f"t{c}")
            nc.gpsimd.dma_start(tt[:, : e - o], tgt3[:, o:e])
            ttiles.append(tt)
            if c == 0:
                nc.sync.dma_start(ematf[:], emat_ap)

        nc.vector.tensor_copy(ematb[:], ematf[:])

        def blocks(w):
            return [(b * MM, min(w, (b + 1) * MM)) for b in range((w + MM - 1) // MM)]

        nblk_total = sum(len(blocks(w)) for w in CHUNKS)
        nblk = 0
        for c in range(NCH):
            w = CHUNKS[c]
            last_chunk = c == NCH - 1
            p_t = ptiles[c][:, :w]
            t_t = ttiles[c][:, :w]

            l1mp = ln_pool.tile([P, CMAX], bf16, tag="l1mp")
            logp = ln_pool.tile([P, CMAX], bf16, tag="logp")
            m2 = mm_pool.tile([P, CMAX], bf16, tag="m2")
            m1 = mm_pool.tile([P, CMAX], bf16, tag="m1")

            def act_l1mp():
                nc.scalar.activation(
                    l1mp[:, :w], p_t, AF.Ln, scale=-1.0, bias=1.0,
                    accum_out=accB[:, c : c + 1],
                )

            def act_logp():
                nc.scalar.activation(logp[:, :w], p_t, AF.Ln)

            # last chunk: logp/m1 path first so the psA chain (the last
            # PSUM fold) closes as early as possible
            if last_chunk:
                act_logp(); act_l1mp()
                nc.vector.tensor_mul(m1[:, :w], t_t, logp[:, :w])
                nc.vector.tensor_mul(m2[:, :w], t_t, l1mp[:, :w])
            else:
                act_l1mp(); act_logp()
                nc.vector.tensor_mul(m2[:, :w], t_t, l1mp[:, :w])
                nc.vector.tensor_mul(m1[:, :w], t_t, logp[:, :w])

            if last_chunk:
                srcs, pss = (t_t, m1[:, :w], m2[:, :w]), (psS, psA, psC)
            else:
                srcs, pss = (t_t, m2[:, :w], m1[:, :w]), (psS, psC, psA)
            for s, e in blocks(w):
                first, last = nblk == 0, nblk == nblk_total - 1
                for src, ps in zip(srcs, pss):
                    nc.tensor.matmul(ps[:, : e - s], ematb[:], src[:, s:e],
                                     start=first, stop=last)
                nblk += 1

        # epilogue: folds split across ACT and DVE so they overlap.
        # ACT: psS, psA via Copy+accum (PSUM reads are cheap on ScE)
        # DVE: accB fold, psC fold, psB copy;  PE: B row-sums matmul
        nc.scalar.activation(junkps[:], psS[:], AF.Copy, accum_out=stats[:, 0:1])
        nc.scalar.activation(junkps[:], psA[:], AF.Copy, accum_out=stats[:, 2:3])
        nc.vector.tensor_reduce(accBr[:], accB[:], axis=mybir.AxisListType.X, op=ALU.add)
        nc.tensor.matmul(psB[:], ematf[:], accBr[:])
        nc.vector.tensor_reduce(stats[:, 3:4], psC[:], axis=mybir.AxisListType.X, op=ALU.add)
        nc.vector.tensor_copy(stats[:, 1:2], psB[:])
        nc.sync.dma_start(out_ap, stats[:])


_PROG_CACHE = {}


def _build_program():
    key = "v5"
    if key not in _PROG_CACHE:
        nc = bacc.Bacc("TRN2", target_bir_lowering=False, debug=False)
        inp = nc.dram_tensor("input", [ROWS, N], f32, kind="ExternalInput").ap()
        tgt = nc.dram_tensor("target", [ROWS, N], f32, kind="ExternalInput").ap()
        emat = nc.dram_tensor("emat", [P, ROWS], f32, kind="ExternalInput").ap()
        out = nc.dram_tensor("partials", [ROWS, OUTW], f32, kind="ExternalOutput").ap()
        with tile.TileContext(nc) as tc:
            _emit(tc, out, inp, tgt, emat)
        nc.finalize()
        _PROG_CACHE[key] = nc
    return _PROG_CACHE[key]


def _emat_np():
    e = np.zeros((P, ROWS), dtype=np.float32)
    for r in range(ROWS):
        e[r * GRP : (r + 1) * GRP, r] = 1.0
    return e


def kernel(input, target):
    global LAST
    input = np.ascontiguousarray(np.asarray(input))
    target = np.ascontiguousarray(np.asarray(target))
    assert input.shape == (B, N) and target.shape == (B, N)

    nc = _build_program()
    emat = _emat_np()
    in_maps = [
        {
            "input": input[c * ROWS : (c + 1) * ROWS],
            "target": target[c * ROWS : (c + 1) * ROWS],
            "emat": emat,
        }
        for c in range(NCORES)
    ]
    res = run_bass_kernel_spmd(nc, in_maps, core_ids=list(range(NCORES)), trace=TRACE)
    LAST = res

    total = np.float64(0.0)
    for c in range(NCORES):
        part = res.results[c]["partials"].astype(np.float64)  # [ROWS, OUTW]
        S, Bv, A, C = part[:, 0], part[:, 1], part[:, 2], part[:, 3]
        beta = 1.0 - S / N
        total += np.sum(beta * A + (1.0 - beta) * (Bv - C))
    return np.float32(-total)
